# revision 28
# baseline (speedup 1.0000x reference)
"""AttentionWithBinding distributed Bass kernel for 8 TRN2 NeuronCores.

Sharding: 8 cores = 2 batches x 4 head-groups (4 heads / 256 dims each).
Per core: q/k/v projections (weight-stationary matmuls from a host
pre-transposed xT), flash-style attention in scoresT [sk, sq] orientation,
softmax exp on ScalarE with the additive binding bias folded in as a
host-precomputed exp(0.5*binding.T) bf16 multiplier on VectorE, row-sums
fused into the attn@v matmul via a ones-column on v, and the per-head
o-projection partials. Host sums the 4 partials per batch and adds the
analytic bias vector bv@Wo + bo.

Structure (vs the naive per-chunk loop):
- One flat software pipeline over all (chunk, head-pair, sk-group)
  slots: scores(i), exp/mul(i-1), attn@v(i-3) with no drain/refill at
  head-pair or chunk boundaries, so neither TensorE nor ScalarE (exp is
  ~36us/chunk) ever starves.
- Input DMAs split between the sync HWDGE ring (consolidated multi-MB
  transfers; ~4us fixed cost per DMA) and the gpsimd SWDGE ring
  (fine-grained 128KB tiles at ~0.7us each), ordered by first use; xT
  arrives column-chunk-first so chunk-0 projections start after ~1.5MB.
  Output staged in SBUF, written as one 1MB DMA per chunk.
- The o-projection of chunk n is deferred and dripped one (row,col)
  piece per pipeline slot through chunk n+1, providing exactly the PE
  filler slack the exp/psum alternation needs (kills HAM re-throttle
  micro-gaps); its evacuations run on VectorE so ScalarE carries only
  the exps (whose chain paces the scores-psum rotation).
- 16 warmup matmuls bridge the initial DMA window to keep the PE clock
  gate warm (the first ~5 run cold, then HAM unthrottles).
"""

import sys

sys.path.insert(0, "/opt/trn_rl_repo")

import numpy as np
import ml_dtypes
from contextlib import ExitStack

BF16 = ml_dtypes.bfloat16

B, S, D = 2, 2048, 1024
H, HD = 16, 64
HPC = 4  # heads per core
DHC = HPC * HD  # 256 head dims per core
SCALE = HD ** -0.5
NCORES = 8
KT = D // 128  # 8 contraction tiles over D
ST = S // 128  # 16 tiles over S
CH = 512  # free-dim chunk (one PSUM bank of f32)
NQ = S // CH  # 4 query chunks

_graph_cache = {}


def _build(has_qk_bias):
    import concourse.bacc as bacc
    import concourse.mybir as mybir
    from concourse import tile

    f32 = mybir.dt.float32
    bf16 = mybir.dt.bfloat16
    AF = mybir.ActivationFunctionType

    nc = bacc.Bacc(None)

    xT_e = nc.declare_dram_parameter("xT", [D, S], bf16, isOutput=False)
    wq_e = nc.declare_dram_parameter("wq", [D, DHC], bf16, isOutput=False)
    wk_e = nc.declare_dram_parameter("wk", [D, DHC], bf16, isOutput=False)
    wv_e = nc.declare_dram_parameter("wv", [D, DHC], bf16, isOutput=False)
    wo_e = nc.declare_dram_parameter("wo", [DHC, D], bf16, isOutput=False)
    eb_e = nc.declare_dram_parameter("expbT", [S, S], bf16, isOutput=False)
    if has_qk_bias:
        bq_e = nc.declare_dram_parameter("bq", [DHC, 1], f32, isOutput=False)
        bk_e = nc.declare_dram_parameter("bk", [DHC, 1], f32, isOutput=False)
    out_e = nc.declare_dram_parameter("out", [S, D], bf16, isOutput=True)

    with tile.TileContext(nc) as tc, ExitStack() as ctx:
        const = ctx.enter_context(tc.tile_pool(name="const", bufs=1))
        xT = const.tile([128, KT, S], bf16)
        eb = const.tile([128, NQ, ST, CH], bf16)  # exp(0.5*binding).T
        wq = const.tile([128, KT, DHC], bf16)
        wk = const.tile([128, KT, DHC], bf16)
        wv = const.tile([128, KT, DHC], bf16)
        junk = const.tile([128, CH], bf16)
        wo = const.tile([128, 2, D], bf16)
        if has_qk_bias:
            bq = const.tile([128, 2], f32)
            bk = const.tile([128, 2], f32)
        qT = const.tile([128, 2, S], bf16)  # [dh, s] head-major
        kT = const.tile([128, 2, S], bf16)
        va = const.tile([128, ST, HPC, 65], bf16)  # v tiles + ones col
        outS = const.tile([128, 2, 4, D], bf16)  # output staging, 2 halves

        nc.vector.memset(junk[:], 0.0)
        # ones columns for the rowsum trick: on the (otherwise idle at
        # start) vector queue so the gpsimd ring is free for DMAs
        for s in range(ST):
            for h in range(HPC):
                nc.vector.memset(va[:, s, h, 64:65], 1.0)

        # The startup is DMA-rate-limited (all 8 cores pull HBM at once:
        # ~70-110GB/s per ring). k0/q0 need only wk+wq+xT cols 0-511, so
        # xT arrives in 0.5MB column chunks split across rings, wq rides
        # at the head of the gpsimd ring, and the weights interleave on
        # the sync ring in first-use order.
        if has_qk_bias:
            for m in range(2):
                nc.sync.dma_start(bq[:, m:m + 1], bq_e[m * 128:(m + 1) * 128, :])
                nc.sync.dma_start(bk[:, m:m + 1], bk_e[m * 128:(m + 1) * 128, :])
        nc.gpsimd.dma_start(
            wq[:], wq_e[:, :].rearrange("(k p) c -> p k c", p=128))
        nc.sync.dma_start(
            wk[:], wk_e[:, :].rearrange("(k p) c -> p k c", p=128))

        def xt_chunk(n):
            nc.sync.dma_start(
                xT[:, 0:4, n * CH:(n + 1) * CH],
                xT_e[0:512, n * CH:(n + 1) * CH].rearrange(
                    "(k p) c -> p k c", p=128))
            nc.gpsimd.dma_start(
                xT[:, 4:8, n * CH:(n + 1) * CH],
                xT_e[512:1024, n * CH:(n + 1) * CH].rearrange(
                    "(k p) c -> p k c", p=128))

        xt_chunk(0)
        xt_chunk(1)
        for t in range(0, 2):
            nc.gpsimd.dma_start(
                eb[:, 0, t, :], eb_e[t * 128:(t + 1) * 128, 0:CH])
        nc.sync.dma_start(
            wv[:], wv_e[:, :].rearrange("(k p) c -> p k c", p=128))
        xt_chunk(2)
        for t in range(2, 4):
            nc.gpsimd.dma_start(
                eb[:, 0, t, :], eb_e[t * 128:(t + 1) * 128, 0:CH])
        xt_chunk(3)
        nc.sync.dma_start(
            wo[:], wo_e[:, :].rearrange("(m p) c -> p m c", p=128))
        for t in range(4, ST):
            nc.gpsimd.dma_start(
                eb[:, 0, t, :], eb_e[t * 128:(t + 1) * 128, 0:CH])
        for n in range(2, NQ):
            nc.sync.dma_start(
                eb[:, n, :, :],
                eb_e[:, n * CH:(n + 1) * CH].rearrange(
                    "(t p) c -> p t c", p=128))
        for t in range(ST):
            nc.gpsimd.dma_start(
                eb[:, 1, t, :], eb_e[t * 128:(t + 1) * 128, CH:2 * CH])

        T2 = 2  # sk tiles merged per exp/mul instruction
        GS = list(range(0, ST, T2)) + [ST]
        NG = len(GS) - 1
        psS = ctx.enter_context(tc.tile_pool(name="psS", bufs=2, space="PSUM"))
        psA = ctx.enter_context(tc.tile_pool(name="psA", bufs=2, space="PSUM"))
        psX = ctx.enter_context(tc.tile_pool(name="psX", bufs=2, space="PSUM"))
        pP = ctx.enter_context(tc.tile_pool(name="pP", bufs=6))
        pP2 = ctx.enter_context(tc.tile_pool(name="pP2", bufs=7))
        pA = ctx.enter_context(tc.tile_pool(name="pA", bufs=2))
        pR = ctx.enter_context(tc.tile_pool(name="pR", bufs=4))
        pRB = ctx.enter_context(tc.tile_pool(name="pRB", bufs=2))

        # dummy matmuls warm the PE clock while the first input DMAs land
        # (~15us for wk/wq + xT chunk 0 at the contended startup rate):
        # the first ~5 run cold (~630ns), then HAM unthrottles
        pw = psX.tile([128, CH], f32, tag="px", name="pw")
        for _ in range(18):
            nc.tensor.matmul(pw[:], junk[:, 0:128], junk[:],
                             start=True, stop=True)

        def qk_proj_m(which, n, m):
            w_t, out_t = (wq, qT) if which == "q" else (wk, kT)
            pp = psX.tile([128, CH], f32, tag="px", name="pp")
            for k in range(KT):
                nc.tensor.matmul(
                    pp[:], w_t[:, k, m * 128:(m + 1) * 128],
                    xT[:, k, n * CH:(n + 1) * CH],
                    start=(k == 0), stop=(k == KT - 1))
            if has_qk_bias:
                b_t = bq if which == "q" else bk
                nc.vector.tensor_scalar_add(
                    out_t[:, m, n * CH:(n + 1) * CH], pp[:],
                    b_t[:, m:m + 1])
            else:
                nc.vector.tensor_copy(
                    out_t[:, m, n * CH:(n + 1) * CH], pp[:])

        def qk_proj_chunk(which, n):
            qk_proj_m(which, n, 0)
            qk_proj_m(which, n, 1)

        def v_proj_tile(s):
            pv = psX.tile([128, HPC, 64], f32, tag="px", name="pv")
            for k in range(KT):
                nc.tensor.matmul(
                    pv[:], xT[:, k, s * 128:(s + 1) * 128], wv[:, k, :],
                    start=(k == 0), stop=(k == KT - 1))
            nc.vector.tensor_copy(va[:, s, :, 0:64], pv[:])

        # upfront: just enough for attention chunk nq=0 to start (scores
        # slot 0 only needs k-chunk 0 + q0; k1..k3 drain as fillers ahead
        # of the sk-groups that read them)
        qk_proj_chunk("k", 0)
        qk_proj_chunk("q", 0)

        # deferred projection work, drained as PE filler inside the
        # attention loop, interleaved to match DMA arrival: k-chunk c is
        # read by scores slot 2(c-... slot 2c-2, v tile 2s+1 by attn@v at
        # slot s+3, so k2/k3 sit between v tiles rather than before them
        # (their xT column chunks land later on the contended rings)
        def K(c, m):
            return lambda: qk_proj_m("k", c, m)

        def V(s):
            return lambda: v_proj_tile(s)

        fillers = [K(1, 0), K(1, 1),
                   V(0), V(1), V(2),
                   K(2, 0), K(2, 1), V(3),
                   V(4), V(5), V(6),
                   K(3, 0), K(3, 1), V(7),
                   V(8), V(9), V(10),
                   V(11), V(12), V(13),
                   V(14), V(15), lambda: qk_proj_m("q", 1, 0),
                   lambda: qk_proj_m("q", 1, 1)]
        fidx = [0]

        def drain_filler(k=1):
            for _ in range(k):
                if fidx[0] < len(fillers):
                    fillers[fidx[0]]()
                    fidx[0] += 1

        # four 512-wide query chunks (a narrower final chunk was tried to
        # shrink the drain, but slots are latency-bound at ~2us regardless
        # of width, so halving per-slot work just halves the efficiency)
        CHUNKS = [(n * CH, CH) for n in range(NQ)]

        def oproj_piece(ci, att, piece, evac="vector"):
            qlo, qw = CHUNKS[ci]
            half = ci % 2
            s4l, dd = divmod(piece, 2)
            s4g = (qlo % CH) // 128 + s4l
            po = psX.tile([128, CH], f32, tag="px", name="po")
            for pr in range(2):
                nc.tensor.matmul(
                    po[:], att[:, pr, s4l * 128:(s4l + 1) * 128],
                    wo[:, pr, dd * CH:(dd + 1) * CH],
                    start=(pr == 0), stop=(pr == 1))
            dst = outS[:, half, s4g, dd * CH:(dd + 1) * CH]
            if evac == "scalar":
                nc.scalar.activation(dst, po[:], AF.Copy)
            else:
                # steady state: all evacuations on VectorE so ScalarE
                # carries only the exps (the exp chain paces the psS
                # rotation; gpsimd can't read PSUM)
                nc.vector.tensor_copy(dst, po[:])

        def oproj_dma(ci):
            qlo, qw = CHUNKS[ci]
            half = ci % 2
            s4g = (qlo % CH) // 128
            nc.sync.dma_start(
                out_e[qlo:qlo + qw, :].rearrange("(s p) c -> p s c", p=128),
                outS[:, half, s4g:s4g + qw // 128, :])

        # flat pipeline over all (chunk, head-pair, sk-group) slots:
        # scores(i), exp/mul(i-1), attn@v(i-3)
        sweeps = [(ci, hp) for ci in range(len(CHUNKS)) for hp in range(2)]
        NS = len(sweeps)
        NSLOT = NS * NG
        sco = {}
        p2s = {}
        accs_by_si = {}
        att_by_ci = {}
        pending_oproj = []
        for slot in range(NSLOT + 3):
            if slot < NSLOT:
                si, g = divmod(slot, NG)
                ci, hp = sweeps[si]
                qlo, qw = CHUNKS[ci]
                if g == 0:
                    if hp == 1:
                        # enqueue the next chunks' q projections one
                        # sweep ahead of first use so they drain well
                        # before the sweep boundary that reads them
                        # (boundary slots are the pipeline's stall point)
                        if qlo == 0:
                            fillers.extend(
                                [lambda m=m: qk_proj_m("q", 2, m)
                                 for m in range(2)])
                        elif qlo == CH:
                            fillers.extend(
                                [lambda m=m: qk_proj_m("q", 3, m)
                                 for m in range(2)])
                    else:
                        att_by_ci[ci] = pA.tile([128, 2, CH], bf16,
                                                name="att", tag="att")
                    accs_by_si[si] = [psA.tile([65, CH], f32, tag="acc",
                                               name=f"acc{j}")
                                      for j in range(2)]
                if slot == 0:
                    drain_filler(2)
                else:
                    drain_filler(3 if slot < 10 else 2)
                sz = GS[g + 1] - GS[g]
                new = [psS.tile([128, T2, CH], f32, tag="sc", name="sc")
                       for j in range(2)]
                for u in range(sz):
                    t = GS[g] + u
                    for j in range(2):
                        nc.tensor.matmul(
                            new[j][:, u, 0:qw],
                            kT[j * 64:(j + 1) * 64, hp,
                               t * 128:(t + 1) * 128],
                            qT[j * 64:(j + 1) * 64, hp, qlo:qlo + qw],
                            start=True, stop=True)
                sco[slot] = new
            if 0 <= slot - 1 < NSLOT:
                sl = slot - 1
                si, g = divmod(sl, NG)
                ci, hp = sweeps[si]
                qlo, qw = CHUNKS[ci]
                nq_c, co = divmod(qlo, CH)
                sz = GS[g + 1] - GS[g]
                cur = []
                for j in range(2):
                    p = pP.tile([128, T2, CH], bf16)
                    nc.scalar.activation(
                        p[:, :sz, 0:qw], sco[sl][j][:, :sz, 0:qw],
                        AF.Exp, scale=SCALE)
                    p2 = pP2.tile([128, T2, CH], bf16)
                    nc.vector.tensor_mul(
                        p2[:, :sz, 0:qw], p[:, :sz, 0:qw],
                        eb[:, nq_c, GS[g]:GS[g] + sz, co:co + qw])
                    cur.append(p2)
                p2s[sl] = cur
                del sco[sl]
            if 0 <= slot - 3 < NSLOT:
                sl = slot - 3
                si, g = divmod(sl, NG)
                ci, hp = sweeps[si]
                qlo, qw = CHUNKS[ci]
                accs = accs_by_si[si]
                sz = GS[g + 1] - GS[g]
                for j in range(2):
                    h = hp * 2 + j
                    for u in range(sz):
                        t = GS[g] + u
                        nc.tensor.matmul(
                            accs[j][:, 0:qw], va[:, t, h, :],
                            p2s[sl][j][:, u, 0:qw],
                            start=(t == 0), stop=(t == ST - 1))
                del p2s[sl]
                if pending_oproj:
                    # one o-projection piece per slot: PE filler spread
                    # through the following sweep, deferred so att is
                    # normalized before these reach the queue head.
                    # (Must finish within one sweep: the att tile's pool
                    # buffer is re-allocated two sweeps later.)
                    oci, oatt, pidx = pending_oproj[0]
                    oproj_piece(oci, oatt, pidx)
                    if pidx == (CHUNKS[oci][1] // 128) * 2 - 1:
                        oproj_dma(oci)
                        pending_oproj.pop(0)
                        del att_by_ci[oci]
                    else:
                        pending_oproj[0] = (oci, oatt, pidx + 1)
                if g == NG - 1:
                    # sweep complete: normalize this head-pair's rows.
                    # (The rowsum must be staged to SBUF first: the
                    # custom-DVE reciprocal's bitwise seed misreads PSUM.)
                    # In the very last sweep ScalarE is idle (no exps
                    # left), so j1's rowsum copy goes there to shorten
                    # the serial drain chain on VectorE.
                    last_sweep = (si == NS - 1)
                    att = att_by_ci[ci]
                    for j in range(2):
                        rs = pR.tile([1, CH], f32, tag="rs", name="rs")
                        if last_sweep and j == 1:
                            nc.scalar.activation(rs[:, 0:qw],
                                                 accs[j][64:65, 0:qw],
                                                 AF.Copy)
                        else:
                            nc.vector.tensor_copy(rs[:, 0:qw],
                                                  accs[j][64:65, 0:qw])
                        r = pR.tile([1, CH], f32)
                        nc.vector.reciprocal_approx_fast(
                            r[:, 0:qw], rs[:, 0:qw])
                        rb = pRB.tile([64, CH], f32)
                        nc.gpsimd.partition_broadcast(
                            rb[:, 0:qw], r[:, 0:qw])
                        nc.vector.tensor_mul(
                            att[j * 64:(j + 1) * 64, hp, 0:qw],
                            accs[j][0:64, 0:qw], rb[:, 0:qw])
                    del accs_by_si[si]
                    if hp == 1:
                        pending_oproj.append((ci, att_by_ci[ci], 0))
        # drain any remaining o-projection (the last half-chunk): both
        # evac engines are idle here, so evacuations alternate
        # scalar/vector to shorten the psX-rotation chain, and each
        # per-s4 output piece leaves immediately on the sync HWDGE ring
        # (drains instantly, unlike the gpsimd SWDGE queue)
        drain_engines = ["scalar", "vector"]
        for oci, oatt, pidx in pending_oproj:
            qlo, qw = CHUNKS[oci]
            half = oci % 2
            s4g0 = (qlo % CH) // 128
            for piece in range(pidx, (qw // 128) * 2):
                oproj_piece(oci, oatt, piece,
                            evac=drain_engines[piece % 2])
                if piece % 2 == 1:
                    s4l = piece // 2
                    nc.sync.dma_start(
                        out_e[qlo + s4l * 128:qlo + (s4l + 1) * 128, :],
                        outS[:, half, s4g0 + s4l, :])
    nc.compile()
    return nc


def _get_graph(has_qk_bias):
    key = ("nc", has_qk_bias)
    if key not in _graph_cache:
        _graph_cache[key] = _build(has_qk_bias)
    return _graph_cache[key]


def _prepare_in_maps(inputs, has_qk_bias):
    x = np.asarray(inputs["x"], np.float32)
    bm = np.asarray(inputs["binding_matrix"], np.float32)
    Wq = np.asarray(inputs["Wq"], np.float32)
    Wk = np.asarray(inputs["Wk"], np.float32)
    Wv = np.asarray(inputs["Wv"], np.float32)
    Wo = np.asarray(inputs["Wo"], np.float32)
    bq = np.asarray(inputs["bq"], np.float32)
    bk = np.asarray(inputs["bk"], np.float32)

    expbT = np.exp(0.5 * bm.T).astype(BF16)
    xTs = [np.ascontiguousarray(x[b].T).astype(BF16) for b in range(B)]
    in_maps = []
    for c in range(NCORES):
        b, g = divmod(c, 4)
        sl = slice(g * DHC, (g + 1) * DHC)
        m = {
            "xT": xTs[b],
            "wq": np.ascontiguousarray(Wq[:, sl]).astype(BF16),
            "wk": np.ascontiguousarray(Wk[:, sl]).astype(BF16),
            "wv": np.ascontiguousarray(Wv[:, sl]).astype(BF16),
            "wo": np.ascontiguousarray(Wo[sl, :]).astype(BF16),
            "expbT": expbT,
        }
        if has_qk_bias:
            m["bq"] = np.ascontiguousarray(bq[sl]).reshape(DHC, 1)
            m["bk"] = np.ascontiguousarray(bk[sl]).reshape(DHC, 1)
        in_maps.append(m)
    return in_maps


def _install_trace_hooks():
    """The container image's antenv stub lacks axon_hooks; synthesize it so
    run_bass_kernel_spmd(trace=True) can reach the NTFF profiler in
    libaxon_pjrt.so, and neuter the bucket artifact upload."""
    import types

    try:
        from antenv.axon_hooks import get_axon_ntff_profile_hook  # noqa: F401
    except ImportError:
        import antenv

        m = types.ModuleType("antenv.axon_hooks")
        m._hook = None
        m.set_axon_ntff_profile_hook = lambda h: setattr(m, "_hook", h)
        m.get_axon_ntff_profile_hook = lambda: m._hook
        sys.modules["antenv.axon_hooks"] = m
        antenv.axon_hooks = m
        if "/root/.axon_site" not in sys.path:
            sys.path.insert(0, "/root/.axon_site")
        from trn_agent_boot.trn_boot import _ntff_profile_via_ctypes

        m._hook = _ntff_profile_via_ctypes("/opt/axon/libaxon_pjrt.so")
    import concourse.bass_utils as bu

    bu.upload_artifacts = lambda tmpdir: str(tmpdir)


def run(inputs, trace=False, tmpdir=None):
    from concourse.bass_utils import run_bass_kernel_spmd

    if trace:
        _install_trace_hooks()
    bq = np.asarray(inputs["bq"], np.float32)
    bk = np.asarray(inputs["bk"], np.float32)
    has_qk_bias = bool(np.any(bq) or np.any(bk))
    nc = _get_graph(has_qk_bias)
    in_maps = _prepare_in_maps(inputs, has_qk_bias)
    res = run_bass_kernel_spmd(nc, in_maps, list(range(NCORES)), trace=trace,
                               tmpdir=tmpdir)

    bv = np.asarray(inputs["bv"], np.float32)
    bo = np.asarray(inputs["bo"], np.float32)
    Wo = np.asarray(inputs["Wo"], np.float32)
    const_vec = (bv @ Wo + bo).astype(np.float32)

    out = np.empty((B, S, D), np.float32)
    for b in range(B):
        acc = np.zeros((S, D), np.float32)
        for g in range(4):
            acc += np.asarray(res.results[b * 4 + g]["out"], np.float32)
        out[b] = acc + const_vec
    return out, res


def kernel(**inputs):
    out, _ = run(inputs, trace=False)
    return out



# revision 29
# speedup vs baseline: 1.0125x; 1.0125x over previous
"""AttentionWithBinding distributed Bass kernel for 8 TRN2 NeuronCores.

Sharding: 8 cores = 2 batches x 4 head-groups (4 heads / 256 dims each).
Per core: q/k/v projections (weight-stationary matmuls from a host
pre-transposed xT), flash-style attention in scoresT [sk, sq] orientation,
softmax exp on ScalarE with the additive binding bias folded in as a
host-precomputed exp(0.5*binding.T) bf16 multiplier on VectorE, row-sums
fused into the attn@v matmul via a ones-column on v, and the per-head
o-projection partials. Host sums the 4 partials per batch and adds the
analytic bias vector bv@Wo + bo.

Structure (vs the naive per-chunk loop):
- One flat software pipeline over all (chunk, head-pair, sk-group)
  slots: scores(i), exp/mul(i-1), attn@v(i-3) with no drain/refill at
  head-pair or chunk boundaries, so neither TensorE nor ScalarE (exp is
  ~36us/chunk) ever starves.
- Input DMAs split between the sync HWDGE ring (consolidated multi-MB
  transfers; ~4us fixed cost per DMA) and the gpsimd SWDGE ring
  (fine-grained 128KB tiles at ~0.7us each), ordered by first use; xT
  arrives column-chunk-first so chunk-0 projections start after ~1.5MB.
  Output staged in SBUF, written as one 1MB DMA per chunk.
- The o-projection of chunk n is deferred and dripped one (row,col)
  piece per pipeline slot through chunk n+1, providing exactly the PE
  filler slack the exp/psum alternation needs (kills HAM re-throttle
  micro-gaps); its evacuations run on VectorE so ScalarE carries only
  the exps (whose chain paces the scores-psum rotation).
- 16 warmup matmuls bridge the initial DMA window to keep the PE clock
  gate warm (the first ~5 run cold, then HAM unthrottles).
"""

import sys

sys.path.insert(0, "/opt/trn_rl_repo")

import numpy as np
import ml_dtypes
from contextlib import ExitStack

BF16 = ml_dtypes.bfloat16

B, S, D = 2, 2048, 1024
H, HD = 16, 64
HPC = 4  # heads per core
DHC = HPC * HD  # 256 head dims per core
SCALE = HD ** -0.5
NCORES = 8
KT = D // 128  # 8 contraction tiles over D
ST = S // 128  # 16 tiles over S
CH = 512  # free-dim chunk (one PSUM bank of f32)
NQ = S // CH  # 4 query chunks

_graph_cache = {}


def _build(has_qk_bias):
    import concourse.bacc as bacc
    import concourse.mybir as mybir
    from concourse import tile

    f32 = mybir.dt.float32
    bf16 = mybir.dt.bfloat16
    AF = mybir.ActivationFunctionType

    nc = bacc.Bacc(None)

    xT_e = nc.declare_dram_parameter("xT", [D, S], bf16, isOutput=False)
    wq_e = nc.declare_dram_parameter("wq", [D, DHC], bf16, isOutput=False)
    wk_e = nc.declare_dram_parameter("wk", [D, DHC], bf16, isOutput=False)
    wv_e = nc.declare_dram_parameter("wv", [D, DHC], bf16, isOutput=False)
    wo_e = nc.declare_dram_parameter("wo", [DHC, D], bf16, isOutput=False)
    eb_e = nc.declare_dram_parameter("expbT", [S, S], bf16, isOutput=False)
    if has_qk_bias:
        bq_e = nc.declare_dram_parameter("bq", [DHC, 1], f32, isOutput=False)
        bk_e = nc.declare_dram_parameter("bk", [DHC, 1], f32, isOutput=False)
    out_e = nc.declare_dram_parameter("out", [S, D], bf16, isOutput=True)

    with tile.TileContext(nc) as tc, ExitStack() as ctx:
        const = ctx.enter_context(tc.tile_pool(name="const", bufs=1))
        xT = const.tile([128, KT, S], bf16)
        eb = const.tile([128, NQ, ST, CH], bf16)  # exp(0.5*binding).T
        wq = const.tile([128, KT, DHC], bf16)
        wk = const.tile([128, KT, DHC], bf16)
        wv = const.tile([128, KT, DHC], bf16)
        junk = const.tile([128, CH], bf16)
        wo = const.tile([128, 2, D], bf16)
        if has_qk_bias:
            bq = const.tile([128, 2], f32)
            bk = const.tile([128, 2], f32)
        qT = const.tile([128, 2, S], bf16)  # [dh, s] head-major
        kT = const.tile([128, 2, S], bf16)
        va = const.tile([128, ST, HPC, 65], bf16)  # v tiles + ones col
        outS = const.tile([128, 2, 4, D], bf16)  # output staging, 2 halves

        nc.vector.memset(junk[:], 0.0)
        # ones columns for the rowsum trick: on the (otherwise idle at
        # start) vector queue so the gpsimd ring is free for DMAs
        for s in range(ST):
            for h in range(HPC):
                nc.vector.memset(va[:, s, h, 64:65], 1.0)

        # The startup is DMA-rate-limited (all 8 cores pull HBM at once:
        # ~70-110GB/s per ring). k0/q0 need only wk+wq+xT cols 0-511, so
        # xT arrives in 0.5MB column chunks split across rings, wq rides
        # at the head of the gpsimd ring, and the weights interleave on
        # the sync ring in first-use order.
        if has_qk_bias:
            for m in range(2):
                nc.sync.dma_start(bq[:, m:m + 1], bq_e[m * 128:(m + 1) * 128, :])
                nc.sync.dma_start(bk[:, m:m + 1], bk_e[m * 128:(m + 1) * 128, :])
        nc.gpsimd.dma_start(
            wq[:], wq_e[:, :].rearrange("(k p) c -> p k c", p=128))
        nc.sync.dma_start(
            wk[:], wk_e[:, :].rearrange("(k p) c -> p k c", p=128))

        def xt_chunk(n):
            nc.sync.dma_start(
                xT[:, 0:4, n * CH:(n + 1) * CH],
                xT_e[0:512, n * CH:(n + 1) * CH].rearrange(
                    "(k p) c -> p k c", p=128))
            nc.gpsimd.dma_start(
                xT[:, 4:8, n * CH:(n + 1) * CH],
                xT_e[512:1024, n * CH:(n + 1) * CH].rearrange(
                    "(k p) c -> p k c", p=128))

        xt_chunk(0)
        xt_chunk(1)
        for t in range(0, 2):
            nc.gpsimd.dma_start(
                eb[:, 0, t, :], eb_e[t * 128:(t + 1) * 128, 0:CH])
        nc.sync.dma_start(
            wv[:], wv_e[:, :].rearrange("(k p) c -> p k c", p=128))
        xt_chunk(2)
        for t in range(2, 4):
            nc.gpsimd.dma_start(
                eb[:, 0, t, :], eb_e[t * 128:(t + 1) * 128, 0:CH])
        xt_chunk(3)
        nc.sync.dma_start(
            wo[:], wo_e[:, :].rearrange("(m p) c -> p m c", p=128))
        for t in range(4, ST):
            nc.gpsimd.dma_start(
                eb[:, 0, t, :], eb_e[t * 128:(t + 1) * 128, 0:CH])
        for n in range(2, NQ):
            nc.sync.dma_start(
                eb[:, n, :, :],
                eb_e[:, n * CH:(n + 1) * CH].rearrange(
                    "(t p) c -> p t c", p=128))
        for t in range(ST):
            nc.gpsimd.dma_start(
                eb[:, 1, t, :], eb_e[t * 128:(t + 1) * 128, CH:2 * CH])

        T2 = 2  # sk tiles merged per exp/mul instruction
        GS = list(range(0, ST, T2)) + [ST]
        NG = len(GS) - 1
        psS = ctx.enter_context(tc.tile_pool(name="psS", bufs=2, space="PSUM"))
        psA = ctx.enter_context(tc.tile_pool(name="psA", bufs=2, space="PSUM"))
        psX = ctx.enter_context(tc.tile_pool(name="psX", bufs=2, space="PSUM"))
        pP = ctx.enter_context(tc.tile_pool(name="pP", bufs=6))
        pP2 = ctx.enter_context(tc.tile_pool(name="pP2", bufs=7))
        pA = ctx.enter_context(tc.tile_pool(name="pA", bufs=2))
        pR = ctx.enter_context(tc.tile_pool(name="pR", bufs=4))
        pRB = ctx.enter_context(tc.tile_pool(name="pRB", bufs=2))

        # dummy matmuls warm the PE clock while the first input DMAs land
        # (~15us for wk/wq + xT chunk 0 at the contended startup rate):
        # the first ~5 run cold (~630ns), then HAM unthrottles
        pw = psX.tile([128, CH], f32, tag="px", name="pw")
        for _ in range(18):
            nc.tensor.matmul(pw[:], junk[:, 0:128], junk[:],
                             start=True, stop=True)

        def qk_proj_m(which, n, m):
            w_t, out_t = (wq, qT) if which == "q" else (wk, kT)
            pp = psX.tile([128, CH], f32, tag="px", name="pp")
            for k in range(KT):
                nc.tensor.matmul(
                    pp[:], w_t[:, k, m * 128:(m + 1) * 128],
                    xT[:, k, n * CH:(n + 1) * CH],
                    start=(k == 0), stop=(k == KT - 1))
            if has_qk_bias:
                b_t = bq if which == "q" else bk
                nc.vector.tensor_scalar_add(
                    out_t[:, m, n * CH:(n + 1) * CH], pp[:],
                    b_t[:, m:m + 1])
            else:
                nc.vector.tensor_copy(
                    out_t[:, m, n * CH:(n + 1) * CH], pp[:])

        def qk_proj_chunk(which, n):
            qk_proj_m(which, n, 0)
            qk_proj_m(which, n, 1)

        def v_proj_tile(s):
            pv = psX.tile([128, HPC, 64], f32, tag="px", name="pv")
            for k in range(KT):
                nc.tensor.matmul(
                    pv[:], xT[:, k, s * 128:(s + 1) * 128], wv[:, k, :],
                    start=(k == 0), stop=(k == KT - 1))
            nc.vector.tensor_copy(va[:, s, :, 0:64], pv[:])

        # upfront: just enough for attention chunk nq=0 to start (scores
        # slot 0 only needs k-chunk 0 + q0; k1..k3 drain as fillers ahead
        # of the sk-groups that read them)
        qk_proj_chunk("k", 0)
        qk_proj_chunk("q", 0)

        # deferred projection work, drained as PE filler inside the
        # attention loop, interleaved to match DMA arrival: k-chunk c is
        # read by scores slot 2(c-... slot 2c-2, v tile 2s+1 by attn@v at
        # slot s+3, so k2/k3 sit between v tiles rather than before them
        # (their xT column chunks land later on the contended rings)
        def K(c, m):
            return lambda: qk_proj_m("k", c, m)

        def V(s):
            return lambda: v_proj_tile(s)

        fillers = [K(1, 0), K(1, 1),
                   V(0), V(1), V(2),
                   K(2, 0), K(2, 1), V(3),
                   V(4), V(5), V(6),
                   K(3, 0), K(3, 1), V(7),
                   V(8), V(9), V(10),
                   V(11), V(12), V(13),
                   V(14), V(15), lambda: qk_proj_m("q", 1, 0),
                   lambda: qk_proj_m("q", 1, 1)]
        fidx = [0]

        def drain_filler(k=1):
            for _ in range(k):
                if fidx[0] < len(fillers):
                    fillers[fidx[0]]()
                    fidx[0] += 1

        # four 512-wide query chunks (a narrower final chunk was tried to
        # shrink the drain, but slots are latency-bound at ~2us regardless
        # of width, so halving per-slot work just halves the efficiency)
        CHUNKS = [(n * CH, CH) for n in range(NQ)]

        def oproj_piece(ci, att, piece, evac="vector"):
            qlo, qw = CHUNKS[ci]
            half = ci % 2
            s4l, dd = divmod(piece, 2)
            s4g = (qlo % CH) // 128 + s4l
            po = psX.tile([128, CH], f32, tag="px", name="po")
            for pr in range(2):
                nc.tensor.matmul(
                    po[:], att[:, pr, s4l * 128:(s4l + 1) * 128],
                    wo[:, pr, dd * CH:(dd + 1) * CH],
                    start=(pr == 0), stop=(pr == 1))
            dst = outS[:, half, s4g, dd * CH:(dd + 1) * CH]
            if evac == "scalar":
                nc.scalar.activation(dst, po[:], AF.Copy)
            else:
                # steady state: all evacuations on VectorE so ScalarE
                # carries only the exps (the exp chain paces the psS
                # rotation; gpsimd can't read PSUM)
                nc.vector.tensor_copy(dst, po[:])

        def oproj_dma(ci):
            qlo, qw = CHUNKS[ci]
            half = ci % 2
            s4g = (qlo % CH) // 128
            nc.sync.dma_start(
                out_e[qlo:qlo + qw, :].rearrange("(s p) c -> p s c", p=128),
                outS[:, half, s4g:s4g + qw // 128, :])

        # flat pipeline over all (chunk, head-pair, sk-group) slots:
        # scores(i), exp/mul(i-1), attn@v(i-3)
        sweeps = [(ci, hp) for ci in range(len(CHUNKS)) for hp in range(2)]
        NS = len(sweeps)
        NSLOT = NS * NG
        sco = {}
        p2s = {}
        accs_by_si = {}
        att_by_ci = {}
        pending_oproj = []
        for slot in range(NSLOT + 3):
            if slot < NSLOT:
                si, g = divmod(slot, NG)
                ci, hp = sweeps[si]
                qlo, qw = CHUNKS[ci]
                if g == 0:
                    if hp == 1:
                        # enqueue the next chunks' q projections one
                        # sweep ahead of first use so they drain well
                        # before the sweep boundary that reads them
                        # (boundary slots are the pipeline's stall point)
                        if qlo == 0:
                            fillers.extend(
                                [lambda m=m: qk_proj_m("q", 2, m)
                                 for m in range(2)])
                        elif qlo == CH:
                            fillers.extend(
                                [lambda m=m: qk_proj_m("q", 3, m)
                                 for m in range(2)])
                    else:
                        att_by_ci[ci] = pA.tile([128, 2, CH], bf16,
                                                name="att", tag="att")
                    accs_by_si[si] = [psA.tile([65, CH], f32, tag="acc",
                                               name=f"acc{j}")
                                      for j in range(2)]
                if slot == 0:
                    drain_filler(2)
                else:
                    drain_filler(3 if slot < 10 else 2)
                sz = GS[g + 1] - GS[g]
                new = [psS.tile([128, T2, CH], f32, tag="sc", name="sc")
                       for j in range(2)]
                # j-major emission: j0's pair finishes two matmuls
                # earlier (j1's completes at the same position), which
                # shortens the scores->exp handoff that paces the
                # ScalarE-saturated sweeps
                for j in range(2):
                    for u in range(sz):
                        t = GS[g] + u
                        nc.tensor.matmul(
                            new[j][:, u, 0:qw],
                            kT[j * 64:(j + 1) * 64, hp,
                               t * 128:(t + 1) * 128],
                            qT[j * 64:(j + 1) * 64, hp, qlo:qlo + qw],
                            start=True, stop=True)
                sco[slot] = new
            if 0 <= slot - 1 < NSLOT:
                sl = slot - 1
                si, g = divmod(sl, NG)
                ci, hp = sweeps[si]
                qlo, qw = CHUNKS[ci]
                nq_c, co = divmod(qlo, CH)
                sz = GS[g + 1] - GS[g]
                cur = []
                for j in range(2):
                    p = pP.tile([128, T2, CH], bf16)
                    nc.scalar.activation(
                        p[:, :sz, 0:qw], sco[sl][j][:, :sz, 0:qw],
                        AF.Exp, scale=SCALE)
                    p2 = pP2.tile([128, T2, CH], bf16)
                    nc.vector.tensor_mul(
                        p2[:, :sz, 0:qw], p[:, :sz, 0:qw],
                        eb[:, nq_c, GS[g]:GS[g] + sz, co:co + qw])
                    cur.append(p2)
                p2s[sl] = cur
                del sco[sl]
            if 0 <= slot - 3 < NSLOT:
                sl = slot - 3
                si, g = divmod(sl, NG)
                ci, hp = sweeps[si]
                qlo, qw = CHUNKS[ci]
                accs = accs_by_si[si]
                sz = GS[g + 1] - GS[g]
                for j in range(2):
                    h = hp * 2 + j
                    for u in range(sz):
                        t = GS[g] + u
                        nc.tensor.matmul(
                            accs[j][:, 0:qw], va[:, t, h, :],
                            p2s[sl][j][:, u, 0:qw],
                            start=(t == 0), stop=(t == ST - 1))
                del p2s[sl]
                if pending_oproj:
                    # one o-projection piece per slot: PE filler spread
                    # through the following sweep, deferred so att is
                    # normalized before these reach the queue head.
                    # (Must finish within one sweep: the att tile's pool
                    # buffer is re-allocated two sweeps later.)
                    oci, oatt, pidx = pending_oproj[0]
                    oproj_piece(oci, oatt, pidx)
                    if pidx == (CHUNKS[oci][1] // 128) * 2 - 1:
                        oproj_dma(oci)
                        pending_oproj.pop(0)
                        del att_by_ci[oci]
                    else:
                        pending_oproj[0] = (oci, oatt, pidx + 1)
                if g == NG - 1:
                    # sweep complete: normalize this head-pair's rows.
                    # (The rowsum must be staged to SBUF first: the
                    # custom-DVE reciprocal's bitwise seed misreads PSUM.)
                    # In the very last sweep ScalarE is idle (no exps
                    # left), so j1's rowsum copy goes there to shorten
                    # the serial drain chain on VectorE.
                    last_sweep = (si == NS - 1)
                    att = att_by_ci[ci]
                    for j in range(2):
                        rs = pR.tile([1, CH], f32, tag="rs", name="rs")
                        if last_sweep and j == 1:
                            nc.scalar.activation(rs[:, 0:qw],
                                                 accs[j][64:65, 0:qw],
                                                 AF.Copy)
                        else:
                            nc.vector.tensor_copy(rs[:, 0:qw],
                                                  accs[j][64:65, 0:qw])
                        r = pR.tile([1, CH], f32)
                        nc.vector.reciprocal_approx_fast(
                            r[:, 0:qw], rs[:, 0:qw])
                        rb = pRB.tile([64, CH], f32)
                        nc.gpsimd.partition_broadcast(
                            rb[:, 0:qw], r[:, 0:qw])
                        nc.vector.tensor_mul(
                            att[j * 64:(j + 1) * 64, hp, 0:qw],
                            accs[j][0:64, 0:qw], rb[:, 0:qw])
                    del accs_by_si[si]
                    if hp == 1:
                        pending_oproj.append((ci, att_by_ci[ci], 0))
        # drain any remaining o-projection (the last half-chunk): both
        # evac engines are idle here, so evacuations alternate
        # scalar/vector to shorten the psX-rotation chain, and each
        # per-s4 output piece leaves immediately on the sync HWDGE ring
        # (drains instantly, unlike the gpsimd SWDGE queue)
        drain_engines = ["scalar", "vector"]
        for oci, oatt, pidx in pending_oproj:
            qlo, qw = CHUNKS[oci]
            half = oci % 2
            s4g0 = (qlo % CH) // 128
            for piece in range(pidx, (qw // 128) * 2):
                oproj_piece(oci, oatt, piece,
                            evac=drain_engines[piece % 2])
                if piece % 2 == 1:
                    s4l = piece // 2
                    nc.sync.dma_start(
                        out_e[qlo + s4l * 128:qlo + (s4l + 1) * 128, :],
                        outS[:, half, s4g0 + s4l, :])
    nc.compile()
    return nc


def _get_graph(has_qk_bias):
    key = ("nc", has_qk_bias)
    if key not in _graph_cache:
        _graph_cache[key] = _build(has_qk_bias)
    return _graph_cache[key]


def _prepare_in_maps(inputs, has_qk_bias):
    x = np.asarray(inputs["x"], np.float32)
    bm = np.asarray(inputs["binding_matrix"], np.float32)
    Wq = np.asarray(inputs["Wq"], np.float32)
    Wk = np.asarray(inputs["Wk"], np.float32)
    Wv = np.asarray(inputs["Wv"], np.float32)
    Wo = np.asarray(inputs["Wo"], np.float32)
    bq = np.asarray(inputs["bq"], np.float32)
    bk = np.asarray(inputs["bk"], np.float32)

    expbT = np.exp(0.5 * bm.T).astype(BF16)
    xTs = [np.ascontiguousarray(x[b].T).astype(BF16) for b in range(B)]
    in_maps = []
    for c in range(NCORES):
        b, g = divmod(c, 4)
        sl = slice(g * DHC, (g + 1) * DHC)
        m = {
            "xT": xTs[b],
            "wq": np.ascontiguousarray(Wq[:, sl]).astype(BF16),
            "wk": np.ascontiguousarray(Wk[:, sl]).astype(BF16),
            "wv": np.ascontiguousarray(Wv[:, sl]).astype(BF16),
            "wo": np.ascontiguousarray(Wo[sl, :]).astype(BF16),
            "expbT": expbT,
        }
        if has_qk_bias:
            m["bq"] = np.ascontiguousarray(bq[sl]).reshape(DHC, 1)
            m["bk"] = np.ascontiguousarray(bk[sl]).reshape(DHC, 1)
        in_maps.append(m)
    return in_maps


def _install_trace_hooks():
    """The container image's antenv stub lacks axon_hooks; synthesize it so
    run_bass_kernel_spmd(trace=True) can reach the NTFF profiler in
    libaxon_pjrt.so, and neuter the bucket artifact upload."""
    import types

    try:
        from antenv.axon_hooks import get_axon_ntff_profile_hook  # noqa: F401
    except ImportError:
        import antenv

        m = types.ModuleType("antenv.axon_hooks")
        m._hook = None
        m.set_axon_ntff_profile_hook = lambda h: setattr(m, "_hook", h)
        m.get_axon_ntff_profile_hook = lambda: m._hook
        sys.modules["antenv.axon_hooks"] = m
        antenv.axon_hooks = m
        if "/root/.axon_site" not in sys.path:
            sys.path.insert(0, "/root/.axon_site")
        from trn_agent_boot.trn_boot import _ntff_profile_via_ctypes

        m._hook = _ntff_profile_via_ctypes("/opt/axon/libaxon_pjrt.so")
    import concourse.bass_utils as bu

    bu.upload_artifacts = lambda tmpdir: str(tmpdir)


def run(inputs, trace=False, tmpdir=None):
    from concourse.bass_utils import run_bass_kernel_spmd

    if trace:
        _install_trace_hooks()
    bq = np.asarray(inputs["bq"], np.float32)
    bk = np.asarray(inputs["bk"], np.float32)
    has_qk_bias = bool(np.any(bq) or np.any(bk))
    nc = _get_graph(has_qk_bias)
    in_maps = _prepare_in_maps(inputs, has_qk_bias)
    res = run_bass_kernel_spmd(nc, in_maps, list(range(NCORES)), trace=trace,
                               tmpdir=tmpdir)

    bv = np.asarray(inputs["bv"], np.float32)
    bo = np.asarray(inputs["bo"], np.float32)
    Wo = np.asarray(inputs["Wo"], np.float32)
    const_vec = (bv @ Wo + bo).astype(np.float32)

    out = np.empty((B, S, D), np.float32)
    for b in range(B):
        acc = np.zeros((S, D), np.float32)
        for g in range(4):
            acc += np.asarray(res.results[b * 4 + g]["out"], np.float32)
        out[b] = acc + const_vec
    return out, res


def kernel(**inputs):
    out, _ = run(inputs, trace=False)
    return out



# revision 30
# speedup vs baseline: 1.0279x; 1.0152x over previous
"""AttentionWithBinding distributed Bass kernel for 8 TRN2 NeuronCores.

Sharding: 8 cores = 2 batches x 4 head-groups (4 heads / 256 dims each).
Per core: q/k/v projections (weight-stationary matmuls from a host
pre-transposed xT), flash-style attention in scoresT [sk, sq] orientation,
softmax exp on ScalarE with the additive binding bias folded in as a
host-precomputed exp(0.5*binding.T) bf16 multiplier on VectorE, row-sums
fused into the attn@v matmul via a ones-column on v, and the per-head
o-projection partials. Host sums the 4 partials per batch and adds the
analytic bias vector bv@Wo + bo.

Structure (vs the naive per-chunk loop):
- One flat software pipeline over all (chunk, head-pair, sk-group)
  slots: scores(i), exp/mul(i-1), attn@v(i-3) with no drain/refill at
  head-pair or chunk boundaries, so neither TensorE nor ScalarE (exp is
  ~36us/chunk) ever starves.
- The startup is DMA-rate-limited (8 cores pull HBM at once, ~70-110
  GB/s per ring): xT arrives in 0.5MB column chunks split across the
  sync HWDGE and gpsimd SWDGE rings, wq rides at the head of the
  gpsimd ring, weights interleave in first-use order, and 18 warmup
  matmuls bridge until k0's inputs land (a smooth idle-free PE ramp
  also avoids HAM re-throttles for the rest of the kernel).
- Mid-kernel sweeps are paced by ScalarE exp saturation (~97% busy):
  the o-projection of chunk n is deferred and dripped one piece per
  slot through chunk n+1 with all evacuations on VectorE, scores are
  emitted j-major to shorten the scores->exp handoff, and the next
  chunk's q projection drains a sweep ahead of the boundary that
  reads it.
- Output staged in SBUF, written as one 1MB DMA per chunk; the final
  chunk drains with scalar/vector-alternating evacuations and per-s4
  pieces on the fast-draining sync ring.
"""

import sys

sys.path.insert(0, "/opt/trn_rl_repo")

import numpy as np
import ml_dtypes
from contextlib import ExitStack

BF16 = ml_dtypes.bfloat16

B, S, D = 2, 2048, 1024
H, HD = 16, 64
HPC = 4  # heads per core
DHC = HPC * HD  # 256 head dims per core
SCALE = HD ** -0.5
NCORES = 8
KT = D // 128  # 8 contraction tiles over D
ST = S // 128  # 16 tiles over S
CH = 512  # free-dim chunk (one PSUM bank of f32)
NQ = S // CH  # 4 query chunks

_graph_cache = {}


def _build(has_qk_bias):
    import concourse.bacc as bacc
    import concourse.mybir as mybir
    from concourse import tile

    f32 = mybir.dt.float32
    bf16 = mybir.dt.bfloat16
    AF = mybir.ActivationFunctionType

    nc = bacc.Bacc(None)

    xT_e = nc.declare_dram_parameter("xT", [D, S], bf16, isOutput=False)
    wq_e = nc.declare_dram_parameter("wq", [D, DHC], bf16, isOutput=False)
    wk_e = nc.declare_dram_parameter("wk", [D, DHC], bf16, isOutput=False)
    wv_e = nc.declare_dram_parameter("wv", [D, DHC], bf16, isOutput=False)
    wo_e = nc.declare_dram_parameter("wo", [DHC, D], bf16, isOutput=False)
    eb_e = nc.declare_dram_parameter("expbT", [S, S], bf16, isOutput=False)
    if has_qk_bias:
        bq_e = nc.declare_dram_parameter("bq", [DHC, 1], f32, isOutput=False)
        bk_e = nc.declare_dram_parameter("bk", [DHC, 1], f32, isOutput=False)
    out_e = nc.declare_dram_parameter("out", [S, D], bf16, isOutput=True)

    with tile.TileContext(nc) as tc, ExitStack() as ctx:
        const = ctx.enter_context(tc.tile_pool(name="const", bufs=1))
        xT = const.tile([128, KT, S], bf16)
        eb = const.tile([128, NQ, ST, CH], bf16)  # exp(0.5*binding).T
        wq = const.tile([128, KT, DHC], bf16)
        wk = const.tile([128, KT, DHC], bf16)
        wv = const.tile([128, KT, DHC], bf16)
        junk = const.tile([128, CH], bf16)
        wo = const.tile([128, 2, D], bf16)
        if has_qk_bias:
            bq = const.tile([128, 2], f32)
            bk = const.tile([128, 2], f32)
        qT = const.tile([128, 2, S], bf16)  # [dh, s] head-major
        kT = const.tile([128, 2, S], bf16)
        va = const.tile([128, ST, HPC, 65], bf16)  # v tiles + ones col
        outS = const.tile([128, 2, 4, D], bf16)  # output staging, 2 halves

        nc.vector.memset(junk[:], 0.0)
        # ones columns for the rowsum trick: on the (otherwise idle at
        # start) vector queue so the gpsimd ring is free for DMAs
        for s in range(ST):
            for h in range(HPC):
                nc.vector.memset(va[:, s, h, 64:65], 1.0)

        # The startup is DMA-rate-limited (all 8 cores pull HBM at once:
        # ~70-110GB/s per ring). k0/q0 need only wk+wq+xT cols 0-511, so
        # xT arrives in 0.5MB column chunks split across rings, wq rides
        # at the head of the gpsimd ring, and the weights interleave on
        # the sync ring in first-use order.
        if has_qk_bias:
            for m in range(2):
                nc.sync.dma_start(bq[:, m:m + 1], bq_e[m * 128:(m + 1) * 128, :])
                nc.sync.dma_start(bk[:, m:m + 1], bk_e[m * 128:(m + 1) * 128, :])
        nc.gpsimd.dma_start(
            wq[:], wq_e[:, :].rearrange("(k p) c -> p k c", p=128))
        nc.sync.dma_start(
            wk[:], wk_e[:, :].rearrange("(k p) c -> p k c", p=128))

        def xt_chunk(n):
            nc.sync.dma_start(
                xT[:, 0:4, n * CH:(n + 1) * CH],
                xT_e[0:512, n * CH:(n + 1) * CH].rearrange(
                    "(k p) c -> p k c", p=128))
            nc.gpsimd.dma_start(
                xT[:, 4:8, n * CH:(n + 1) * CH],
                xT_e[512:1024, n * CH:(n + 1) * CH].rearrange(
                    "(k p) c -> p k c", p=128))

        xt_chunk(0)
        xt_chunk(1)
        for t in range(0, 2):
            nc.gpsimd.dma_start(
                eb[:, 0, t, :], eb_e[t * 128:(t + 1) * 128, 0:CH])
        nc.sync.dma_start(
            wv[:], wv_e[:, :].rearrange("(k p) c -> p k c", p=128))
        xt_chunk(2)
        for t in range(2, 4):
            nc.gpsimd.dma_start(
                eb[:, 0, t, :], eb_e[t * 128:(t + 1) * 128, 0:CH])
        xt_chunk(3)
        nc.sync.dma_start(
            wo[:], wo_e[:, :].rearrange("(m p) c -> p m c", p=128))
        for t in range(4, ST):
            nc.gpsimd.dma_start(
                eb[:, 0, t, :], eb_e[t * 128:(t + 1) * 128, 0:CH])
        for n in range(2, NQ):
            nc.sync.dma_start(
                eb[:, n, :, :],
                eb_e[:, n * CH:(n + 1) * CH].rearrange(
                    "(t p) c -> p t c", p=128))
        for t in range(ST):
            nc.gpsimd.dma_start(
                eb[:, 1, t, :], eb_e[t * 128:(t + 1) * 128, CH:2 * CH])

        T2 = 2  # sk tiles merged per exp/mul instruction
        GS = list(range(0, ST, T2)) + [ST]
        NG = len(GS) - 1
        psS = ctx.enter_context(tc.tile_pool(name="psS", bufs=2, space="PSUM"))
        psA = ctx.enter_context(tc.tile_pool(name="psA", bufs=2, space="PSUM"))
        psX = ctx.enter_context(tc.tile_pool(name="psX", bufs=2, space="PSUM"))
        pP = ctx.enter_context(tc.tile_pool(name="pP", bufs=6))
        pP2 = ctx.enter_context(tc.tile_pool(name="pP2", bufs=7))
        pA = ctx.enter_context(tc.tile_pool(name="pA", bufs=2))
        pR = ctx.enter_context(tc.tile_pool(name="pR", bufs=4))
        pRB = ctx.enter_context(tc.tile_pool(name="pRB", bufs=2))

        # dummy matmuls warm the PE clock while the first input DMAs land
        # (~15us for wk/wq + xT chunk 0 at the contended startup rate):
        # the first ~5 run cold (~630ns), then HAM unthrottles
        pw = psX.tile([128, CH], f32, tag="px", name="pw")
        for _ in range(18):
            nc.tensor.matmul(pw[:], junk[:, 0:128], junk[:],
                             start=True, stop=True)

        def qk_proj_m(which, n, m):
            w_t, out_t = (wq, qT) if which == "q" else (wk, kT)
            pp = psX.tile([128, CH], f32, tag="px", name="pp")
            for k in range(KT):
                nc.tensor.matmul(
                    pp[:], w_t[:, k, m * 128:(m + 1) * 128],
                    xT[:, k, n * CH:(n + 1) * CH],
                    start=(k == 0), stop=(k == KT - 1))
            if has_qk_bias:
                b_t = bq if which == "q" else bk
                nc.vector.tensor_scalar_add(
                    out_t[:, m, n * CH:(n + 1) * CH], pp[:],
                    b_t[:, m:m + 1])
            else:
                nc.vector.tensor_copy(
                    out_t[:, m, n * CH:(n + 1) * CH], pp[:])

        def qk_proj_chunk(which, n):
            qk_proj_m(which, n, 0)
            qk_proj_m(which, n, 1)

        def v_proj_tile(s):
            pv = psX.tile([128, HPC, 64], f32, tag="px", name="pv")
            for k in range(KT):
                nc.tensor.matmul(
                    pv[:], xT[:, k, s * 128:(s + 1) * 128], wv[:, k, :],
                    start=(k == 0), stop=(k == KT - 1))
            nc.vector.tensor_copy(va[:, s, :, 0:64], pv[:])

        # upfront: just enough for attention chunk nq=0 to start (scores
        # slot 0 only needs k-chunk 0 + q0; k1..k3 drain as fillers ahead
        # of the sk-groups that read them)
        qk_proj_chunk("k", 0)
        qk_proj_chunk("q", 0)

        # deferred projection work, drained as PE filler inside the
        # attention loop, interleaved to match DMA arrival: k-chunk c is
        # read by scores slot 2(c-... slot 2c-2, v tile 2s+1 by attn@v at
        # slot s+3, so k2/k3 sit between v tiles rather than before them
        # (their xT column chunks land later on the contended rings)
        def K(c, m):
            return lambda: qk_proj_m("k", c, m)

        def V(s):
            return lambda: v_proj_tile(s)

        fillers = [K(1, 0), K(1, 1),
                   V(0), V(1), V(2),
                   K(2, 0), K(2, 1), V(3),
                   V(4), V(5), V(6),
                   K(3, 0), K(3, 1), V(7),
                   V(8), V(9), V(10),
                   V(11), V(12), V(13),
                   V(14), V(15), lambda: qk_proj_m("q", 1, 0),
                   lambda: qk_proj_m("q", 1, 1)]
        fidx = [0]

        def drain_filler(k=1):
            for _ in range(k):
                if fidx[0] < len(fillers):
                    fillers[fidx[0]]()
                    fidx[0] += 1

        # four 512-wide query chunks (a narrower final chunk was tried to
        # shrink the drain, but slots are latency-bound at ~2us regardless
        # of width, so halving per-slot work just halves the efficiency)
        CHUNKS = [(n * CH, CH) for n in range(NQ)]

        def oproj_piece(ci, att, piece, evac="vector"):
            qlo, qw = CHUNKS[ci]
            half = ci % 2
            s4l, dd = divmod(piece, 2)
            s4g = (qlo % CH) // 128 + s4l
            po = psX.tile([128, CH], f32, tag="px", name="po")
            for pr in range(2):
                nc.tensor.matmul(
                    po[:], att[:, pr, s4l * 128:(s4l + 1) * 128],
                    wo[:, pr, dd * CH:(dd + 1) * CH],
                    start=(pr == 0), stop=(pr == 1))
            dst = outS[:, half, s4g, dd * CH:(dd + 1) * CH]
            if evac == "scalar":
                nc.scalar.activation(dst, po[:], AF.Copy)
            else:
                # steady state: all evacuations on VectorE so ScalarE
                # carries only the exps (the exp chain paces the psS
                # rotation; gpsimd can't read PSUM)
                nc.vector.tensor_copy(dst, po[:])

        def oproj_dma(ci):
            qlo, qw = CHUNKS[ci]
            half = ci % 2
            s4g = (qlo % CH) // 128
            nc.sync.dma_start(
                out_e[qlo:qlo + qw, :].rearrange("(s p) c -> p s c", p=128),
                outS[:, half, s4g:s4g + qw // 128, :])

        # flat pipeline over all (chunk, head-pair, sk-group) slots:
        # scores(i), exp/mul(i-1), attn@v(i-3)
        sweeps = [(ci, hp) for ci in range(len(CHUNKS)) for hp in range(2)]
        NS = len(sweeps)
        NSLOT = NS * NG
        sco = {}
        p2s = {}
        accs_by_si = {}
        att_by_ci = {}
        pending_oproj = []
        for slot in range(NSLOT + 3):
            if slot < NSLOT:
                si, g = divmod(slot, NG)
                ci, hp = sweeps[si]
                qlo, qw = CHUNKS[ci]
                if g == 0:
                    if hp == 1:
                        # enqueue the next chunks' q projections one
                        # sweep ahead of first use so they drain well
                        # before the sweep boundary that reads them
                        # (boundary slots are the pipeline's stall point)
                        if qlo == 0:
                            fillers.extend(
                                [lambda m=m: qk_proj_m("q", 2, m)
                                 for m in range(2)])
                        elif qlo == CH:
                            fillers.extend(
                                [lambda m=m: qk_proj_m("q", 3, m)
                                 for m in range(2)])
                    else:
                        att_by_ci[ci] = pA.tile([128, 2, CH], bf16,
                                                name="att", tag="att")
                    accs_by_si[si] = [psA.tile([65, CH], f32, tag="acc",
                                               name=f"acc{j}")
                                      for j in range(2)]
                if slot == 0:
                    drain_filler(2)
                else:
                    drain_filler(3 if slot < 10 else 2)
                sz = GS[g + 1] - GS[g]
                new = [psS.tile([128, T2, CH], f32, tag="sc", name="sc")
                       for j in range(2)]
                # j-major emission: j0's pair finishes two matmuls
                # earlier (j1's completes at the same position), which
                # shortens the scores->exp handoff that paces the
                # ScalarE-saturated sweeps
                for j in range(2):
                    for u in range(sz):
                        t = GS[g] + u
                        nc.tensor.matmul(
                            new[j][:, u, 0:qw],
                            kT[j * 64:(j + 1) * 64, hp,
                               t * 128:(t + 1) * 128],
                            qT[j * 64:(j + 1) * 64, hp, qlo:qlo + qw],
                            start=True, stop=True)
                sco[slot] = new
            if 0 <= slot - 1 < NSLOT:
                sl = slot - 1
                si, g = divmod(sl, NG)
                ci, hp = sweeps[si]
                qlo, qw = CHUNKS[ci]
                nq_c, co = divmod(qlo, CH)
                sz = GS[g + 1] - GS[g]
                cur = []
                for j in range(2):
                    p = pP.tile([128, T2, CH], bf16)
                    nc.scalar.activation(
                        p[:, :sz, 0:qw], sco[sl][j][:, :sz, 0:qw],
                        AF.Exp, scale=SCALE)
                    p2 = pP2.tile([128, T2, CH], bf16)
                    nc.vector.tensor_mul(
                        p2[:, :sz, 0:qw], p[:, :sz, 0:qw],
                        eb[:, nq_c, GS[g]:GS[g] + sz, co:co + qw])
                    cur.append(p2)
                p2s[sl] = cur
                del sco[sl]
            if 0 <= slot - 3 < NSLOT:
                sl = slot - 3
                si, g = divmod(sl, NG)
                ci, hp = sweeps[si]
                qlo, qw = CHUNKS[ci]
                accs = accs_by_si[si]
                sz = GS[g + 1] - GS[g]
                for j in range(2):
                    h = hp * 2 + j
                    for u in range(sz):
                        t = GS[g] + u
                        nc.tensor.matmul(
                            accs[j][:, 0:qw], va[:, t, h, :],
                            p2s[sl][j][:, u, 0:qw],
                            start=(t == 0), stop=(t == ST - 1))
                del p2s[sl]
                if pending_oproj:
                    # one o-projection piece per slot: PE filler spread
                    # through the following sweep, deferred so att is
                    # normalized before these reach the queue head.
                    # (Must finish within one sweep: the att tile's pool
                    # buffer is re-allocated two sweeps later.)
                    oci, oatt, pidx = pending_oproj[0]
                    oproj_piece(oci, oatt, pidx)
                    if pidx == (CHUNKS[oci][1] // 128) * 2 - 1:
                        oproj_dma(oci)
                        pending_oproj.pop(0)
                        del att_by_ci[oci]
                    else:
                        pending_oproj[0] = (oci, oatt, pidx + 1)
                if g == NG - 1:
                    # sweep complete: normalize this head-pair's rows.
                    # (The rowsum must be staged to SBUF first: the
                    # custom-DVE reciprocal's bitwise seed misreads PSUM.)
                    # In the very last sweep ScalarE is idle (no exps
                    # left), so j1's rowsum copy goes there to shorten
                    # the serial drain chain on VectorE.
                    last_sweep = (si == NS - 1)
                    att = att_by_ci[ci]
                    for j in range(2):
                        rs = pR.tile([1, CH], f32, tag="rs", name="rs")
                        if last_sweep and j == 1:
                            nc.scalar.activation(rs[:, 0:qw],
                                                 accs[j][64:65, 0:qw],
                                                 AF.Copy)
                        else:
                            nc.vector.tensor_copy(rs[:, 0:qw],
                                                  accs[j][64:65, 0:qw])
                        r = pR.tile([1, CH], f32)
                        nc.vector.reciprocal_approx_fast(
                            r[:, 0:qw], rs[:, 0:qw])
                        rb = pRB.tile([64, CH], f32)
                        nc.gpsimd.partition_broadcast(
                            rb[:, 0:qw], r[:, 0:qw])
                        nc.vector.tensor_mul(
                            att[j * 64:(j + 1) * 64, hp, 0:qw],
                            accs[j][0:64, 0:qw], rb[:, 0:qw])
                    del accs_by_si[si]
                    if hp == 1:
                        pending_oproj.append((ci, att_by_ci[ci], 0))
        # drain any remaining o-projection (the last half-chunk): both
        # evac engines are idle here, so evacuations alternate
        # scalar/vector to shorten the psX-rotation chain, and each
        # per-s4 output piece leaves immediately on the sync HWDGE ring
        # (drains instantly, unlike the gpsimd SWDGE queue)
        drain_engines = ["scalar", "vector"]
        for oci, oatt, pidx in pending_oproj:
            qlo, qw = CHUNKS[oci]
            half = oci % 2
            s4g0 = (qlo % CH) // 128
            for piece in range(pidx, (qw // 128) * 2):
                oproj_piece(oci, oatt, piece,
                            evac=drain_engines[piece % 2])
                if piece % 2 == 1:
                    s4l = piece // 2
                    nc.sync.dma_start(
                        out_e[qlo + s4l * 128:qlo + (s4l + 1) * 128, :],
                        outS[:, half, s4g0 + s4l, :])
    nc.compile()
    return nc


def _get_graph(has_qk_bias):
    key = ("nc", has_qk_bias)
    if key not in _graph_cache:
        _graph_cache[key] = _build(has_qk_bias)
    return _graph_cache[key]


def _prepare_in_maps(inputs, has_qk_bias):
    x = np.asarray(inputs["x"], np.float32)
    bm = np.asarray(inputs["binding_matrix"], np.float32)
    Wq = np.asarray(inputs["Wq"], np.float32)
    Wk = np.asarray(inputs["Wk"], np.float32)
    Wv = np.asarray(inputs["Wv"], np.float32)
    Wo = np.asarray(inputs["Wo"], np.float32)
    bq = np.asarray(inputs["bq"], np.float32)
    bk = np.asarray(inputs["bk"], np.float32)

    expbT = np.exp(0.5 * bm.T).astype(BF16)
    xTs = [np.ascontiguousarray(x[b].T).astype(BF16) for b in range(B)]
    in_maps = []
    for c in range(NCORES):
        b, g = divmod(c, 4)
        sl = slice(g * DHC, (g + 1) * DHC)
        m = {
            "xT": xTs[b],
            "wq": np.ascontiguousarray(Wq[:, sl]).astype(BF16),
            "wk": np.ascontiguousarray(Wk[:, sl]).astype(BF16),
            "wv": np.ascontiguousarray(Wv[:, sl]).astype(BF16),
            "wo": np.ascontiguousarray(Wo[sl, :]).astype(BF16),
            "expbT": expbT,
        }
        if has_qk_bias:
            m["bq"] = np.ascontiguousarray(bq[sl]).reshape(DHC, 1)
            m["bk"] = np.ascontiguousarray(bk[sl]).reshape(DHC, 1)
        in_maps.append(m)
    return in_maps


def _install_trace_hooks():
    """The container image's antenv stub lacks axon_hooks; synthesize it so
    run_bass_kernel_spmd(trace=True) can reach the NTFF profiler in
    libaxon_pjrt.so, and neuter the bucket artifact upload."""
    import types

    try:
        from antenv.axon_hooks import get_axon_ntff_profile_hook  # noqa: F401
    except ImportError:
        import antenv

        m = types.ModuleType("antenv.axon_hooks")
        m._hook = None
        m.set_axon_ntff_profile_hook = lambda h: setattr(m, "_hook", h)
        m.get_axon_ntff_profile_hook = lambda: m._hook
        sys.modules["antenv.axon_hooks"] = m
        antenv.axon_hooks = m
        if "/root/.axon_site" not in sys.path:
            sys.path.insert(0, "/root/.axon_site")
        from trn_agent_boot.trn_boot import _ntff_profile_via_ctypes

        m._hook = _ntff_profile_via_ctypes("/opt/axon/libaxon_pjrt.so")
    import concourse.bass_utils as bu

    bu.upload_artifacts = lambda tmpdir: str(tmpdir)


def run(inputs, trace=False, tmpdir=None):
    from concourse.bass_utils import run_bass_kernel_spmd

    if trace:
        _install_trace_hooks()
    bq = np.asarray(inputs["bq"], np.float32)
    bk = np.asarray(inputs["bk"], np.float32)
    has_qk_bias = bool(np.any(bq) or np.any(bk))
    nc = _get_graph(has_qk_bias)
    in_maps = _prepare_in_maps(inputs, has_qk_bias)
    res = run_bass_kernel_spmd(nc, in_maps, list(range(NCORES)), trace=trace,
                               tmpdir=tmpdir)

    bv = np.asarray(inputs["bv"], np.float32)
    bo = np.asarray(inputs["bo"], np.float32)
    Wo = np.asarray(inputs["Wo"], np.float32)
    const_vec = (bv @ Wo + bo).astype(np.float32)

    out = np.empty((B, S, D), np.float32)
    for b in range(B):
        acc = np.zeros((S, D), np.float32)
        for g in range(4):
            acc += np.asarray(res.results[b * 4 + g]["out"], np.float32)
        out[b] = acc + const_vec
    return out, res


def kernel(**inputs):
    out, _ = run(inputs, trace=False)
    return out



# revision 34
# speedup vs baseline: 1.0514x; 1.0228x over previous
"""AttentionWithBinding distributed Bass kernel for 8 TRN2 NeuronCores.

Sharding: 8 cores = 2 batches x 4 head-groups (4 heads / 256 dims each).
Per core: q/k/v projections (weight-stationary matmuls from a host
pre-transposed xT), flash-style attention in scoresT [sk, sq] orientation,
softmax exp on ScalarE with the additive binding bias folded in as a
host-precomputed exp(0.5*binding.T) bf16 multiplier on VectorE, row-sums
fused into the attn@v matmul via a ones-column on v, and the per-head
o-projection partials. Host sums the 4 partials per batch and adds the
analytic bias vector bv@Wo + bo.

Structure (vs the naive per-chunk loop):
- One flat software pipeline over all (chunk, head-pair, sk-group)
  slots: scores(i), exp/mul(i-1), attn@v(i-3) with no drain/refill at
  head-pair or chunk boundaries, so neither TensorE nor ScalarE (exp is
  ~36us/chunk) ever starves.
- The startup is DMA-rate-limited (8 cores pull HBM at once, ~70-110
  GB/s per ring): xT arrives in 0.5MB column chunks split across the
  sync HWDGE and gpsimd SWDGE rings, wq rides at the head of the
  gpsimd ring, weights interleave in first-use order, and 18 warmup
  matmuls bridge until k0's inputs land (a smooth idle-free PE ramp
  also avoids HAM re-throttles for the rest of the kernel).
- Mid-kernel sweeps are paced by ScalarE exp saturation (~97% busy):
  the o-projection of chunk n is deferred and dripped one piece per
  slot through chunk n+1 with all evacuations on VectorE, scores are
  emitted j-major to shorten the scores->exp handoff, and the next
  chunk's q projection drains a sweep ahead of the boundary that
  reads it.
- Output staged in SBUF, written as one 1MB DMA per chunk; the final
  chunk drains with scalar/vector-alternating evacuations and per-s4
  pieces on the fast-draining sync ring.
"""

import sys

sys.path.insert(0, "/opt/trn_rl_repo")

import numpy as np
import ml_dtypes
from contextlib import ExitStack

BF16 = ml_dtypes.bfloat16

B, S, D = 2, 2048, 1024
H, HD = 16, 64
HPC = 4  # heads per core
DHC = HPC * HD  # 256 head dims per core
SCALE = HD ** -0.5
NCORES = 8
KT = D // 128  # 8 contraction tiles over D
ST = S // 128  # 16 tiles over S
CH = 512  # free-dim chunk (one PSUM bank of f32)
NQ = S // CH  # 4 query chunks

_graph_cache = {}


def _build(has_qk_bias):
    import concourse.bacc as bacc
    import concourse.mybir as mybir
    from concourse import tile

    f32 = mybir.dt.float32
    bf16 = mybir.dt.bfloat16
    AF = mybir.ActivationFunctionType

    nc = bacc.Bacc(None)

    xT_e = nc.declare_dram_parameter("xT", [D, S], bf16, isOutput=False)
    wq_e = nc.declare_dram_parameter("wq", [D, DHC], bf16, isOutput=False)
    wk_e = nc.declare_dram_parameter("wk", [D, DHC], bf16, isOutput=False)
    wv_e = nc.declare_dram_parameter("wv", [D, DHC], bf16, isOutput=False)
    wo_e = nc.declare_dram_parameter("wo", [DHC, D], bf16, isOutput=False)
    eb_e = nc.declare_dram_parameter("expbT", [S, S], bf16, isOutput=False)
    if has_qk_bias:
        bq_e = nc.declare_dram_parameter("bq", [DHC, 1], f32, isOutput=False)
        bk_e = nc.declare_dram_parameter("bk", [DHC, 1], f32, isOutput=False)
    out_e = nc.declare_dram_parameter("out", [S, D], bf16, isOutput=True)

    with tile.TileContext(nc) as tc, ExitStack() as ctx:
        const = ctx.enter_context(tc.tile_pool(name="const", bufs=1))
        xT = const.tile([128, KT, S], bf16)
        eb = const.tile([128, NQ, ST, CH], bf16)  # exp(0.5*binding).T
        wq = const.tile([128, KT, DHC], bf16)
        wk = const.tile([128, KT, DHC], bf16)
        wv = const.tile([128, KT, DHC], bf16)
        junk = const.tile([128, CH], bf16)
        wo = const.tile([128, 2, D], bf16)
        if has_qk_bias:
            bq = const.tile([128, 2], f32)
            bk = const.tile([128, 2], f32)
        qT = const.tile([128, 2, S], bf16)  # [dh, s] head-major
        kT = const.tile([128, 2, S], bf16)
        va = const.tile([128, ST, HPC, 65], bf16)  # v tiles + ones col
        outS = const.tile([128, 2, 4, D], bf16)  # output staging, 2 halves

        nc.vector.memset(junk[:], 0.0)
        # ones columns for the rowsum trick: on the (otherwise idle at
        # start) vector queue so the gpsimd ring is free for DMAs
        for s in range(ST):
            for h in range(HPC):
                nc.vector.memset(va[:, s, h, 64:65], 1.0)

        # The startup is DMA-rate-limited (all 8 cores pull HBM at once:
        # ~70-110GB/s per ring). k0/q0 need only wk+wq+xT cols 0-511, so
        # xT arrives in 0.5MB column chunks split across rings, wq rides
        # at the head of the gpsimd ring, and the weights interleave on
        # the sync ring in first-use order.
        if has_qk_bias:
            for m in range(2):
                nc.sync.dma_start(bq[:, m:m + 1], bq_e[m * 128:(m + 1) * 128, :])
                nc.sync.dma_start(bk[:, m:m + 1], bk_e[m * 128:(m + 1) * 128, :])
        nc.gpsimd.dma_start(
            wq[:], wq_e[:, :].rearrange("(k p) c -> p k c", p=128))
        nc.sync.dma_start(
            wk[:], wk_e[:, :].rearrange("(k p) c -> p k c", p=128))

        def xt_chunk(n):
            nc.sync.dma_start(
                xT[:, 0:4, n * CH:(n + 1) * CH],
                xT_e[0:512, n * CH:(n + 1) * CH].rearrange(
                    "(k p) c -> p k c", p=128))
            nc.gpsimd.dma_start(
                xT[:, 4:8, n * CH:(n + 1) * CH],
                xT_e[512:1024, n * CH:(n + 1) * CH].rearrange(
                    "(k p) c -> p k c", p=128))

        xt_chunk(0)
        xt_chunk(1)
        for t in range(0, 2):
            nc.gpsimd.dma_start(
                eb[:, 0, t, :], eb_e[t * 128:(t + 1) * 128, 0:CH])
        nc.sync.dma_start(
            wv[:], wv_e[:, :].rearrange("(k p) c -> p k c", p=128))
        xt_chunk(2)
        for t in range(2, 4):
            nc.gpsimd.dma_start(
                eb[:, 0, t, :], eb_e[t * 128:(t + 1) * 128, 0:CH])
        xt_chunk(3)
        nc.sync.dma_start(
            wo[:], wo_e[:, :].rearrange("(m p) c -> p m c", p=128))
        for t in range(4, ST):
            nc.gpsimd.dma_start(
                eb[:, 0, t, :], eb_e[t * 128:(t + 1) * 128, 0:CH])
        for n in range(2, NQ):
            nc.sync.dma_start(
                eb[:, n, :, :],
                eb_e[:, n * CH:(n + 1) * CH].rearrange(
                    "(t p) c -> p t c", p=128))
        for t in range(ST):
            nc.gpsimd.dma_start(
                eb[:, 1, t, :], eb_e[t * 128:(t + 1) * 128, CH:2 * CH])

        T2 = 2  # sk tiles merged per exp/mul instruction
        GS = list(range(0, ST, T2)) + [ST]
        NG = len(GS) - 1
        psS = ctx.enter_context(tc.tile_pool(name="psS", bufs=2, space="PSUM"))
        psA = ctx.enter_context(tc.tile_pool(name="psA", bufs=2, space="PSUM"))
        psX = ctx.enter_context(tc.tile_pool(name="psX", bufs=2, space="PSUM"))
        pP = ctx.enter_context(tc.tile_pool(name="pP", bufs=6))
        pP2 = ctx.enter_context(tc.tile_pool(name="pP2", bufs=7))
        pA = ctx.enter_context(tc.tile_pool(name="pA", bufs=2))
        pR = ctx.enter_context(tc.tile_pool(name="pR", bufs=4))
        pRB = ctx.enter_context(tc.tile_pool(name="pRB", bufs=2))

        # dummy matmuls warm the PE clock while the first input DMAs land
        # (~15us for wk/wq + xT chunk 0 at the contended startup rate):
        # the first ~5 run cold (~630ns), then HAM unthrottles
        pw = psX.tile([128, CH], f32, tag="px", name="pw")
        for _ in range(18):
            nc.tensor.matmul(pw[:], junk[:, 0:128], junk[:],
                             start=True, stop=True)

        def qk_proj_m(which, n, m):
            w_t, out_t = (wq, qT) if which == "q" else (wk, kT)
            pp = psX.tile([128, CH], f32, tag="px", name="pp")
            for k in range(KT):
                nc.tensor.matmul(
                    pp[:], w_t[:, k, m * 128:(m + 1) * 128],
                    xT[:, k, n * CH:(n + 1) * CH],
                    start=(k == 0), stop=(k == KT - 1))
            if has_qk_bias:
                b_t = bq if which == "q" else bk
                nc.vector.tensor_scalar_add(
                    out_t[:, m, n * CH:(n + 1) * CH], pp[:],
                    b_t[:, m:m + 1])
            else:
                nc.vector.tensor_copy(
                    out_t[:, m, n * CH:(n + 1) * CH], pp[:])

        def qk_proj_chunk(which, n):
            qk_proj_m(which, n, 0)
            qk_proj_m(which, n, 1)

        def v_proj_tile(s):
            pv = psX.tile([128, HPC, 64], f32, tag="px", name="pv")
            for k in range(KT):
                nc.tensor.matmul(
                    pv[:], xT[:, k, s * 128:(s + 1) * 128], wv[:, k, :],
                    start=(k == 0), stop=(k == KT - 1))
            nc.vector.tensor_copy(va[:, s, :, 0:64], pv[:])

        # upfront: just enough for attention chunk nq=0 to start (scores
        # slot 0 only needs k-chunk 0 + q0; k1..k3 drain as fillers ahead
        # of the sk-groups that read them)
        qk_proj_chunk("k", 0)
        qk_proj_chunk("q", 0)

        # deferred projection work, drained as PE filler inside the
        # attention loop, interleaved to match DMA arrival: k-chunk c is
        # read by scores slot 2(c-... slot 2c-2, v tile 2s+1 by attn@v at
        # slot s+3, so k2/k3 sit between v tiles rather than before them
        # (their xT column chunks land later on the contended rings)
        def K(c, m):
            return lambda: qk_proj_m("k", c, m)

        def V(s):
            return lambda: v_proj_tile(s)

        fillers = [K(1, 0), K(1, 1),
                   V(0), V(1), V(2),
                   K(2, 0), K(2, 1), V(3),
                   V(4), V(5), V(6),
                   K(3, 0), K(3, 1), V(7),
                   V(8), V(9), V(10),
                   V(11), V(12), V(13),
                   V(14), V(15), lambda: qk_proj_m("q", 1, 0),
                   lambda: qk_proj_m("q", 1, 1)]
        fidx = [0]

        def drain_filler(k=1):
            for _ in range(k):
                if fidx[0] < len(fillers):
                    fillers[fidx[0]]()
                    fidx[0] += 1

        # four 512-wide query chunks (a narrower final chunk was tried to
        # shrink the drain, but slots are latency-bound at ~2us regardless
        # of width, so halving per-slot work just halves the efficiency)
        CHUNKS = [(n * CH, CH) for n in range(NQ)]

        def oproj_piece(ci, att, piece, evac="vector", po=None):
            qlo, qw = CHUNKS[ci]
            half = ci % 2
            s4l, dd = divmod(piece, 2)
            s4g = (qlo % CH) // 128 + s4l
            if po is None:
                po = psX.tile([128, CH], f32, tag="px", name="po")
            for pr in range(2):
                nc.tensor.matmul(
                    po[:], att[:, pr, s4l * 128:(s4l + 1) * 128],
                    wo[:, pr, dd * CH:(dd + 1) * CH],
                    start=(pr == 0), stop=(pr == 1))
            dst = outS[:, half, s4g, dd * CH:(dd + 1) * CH]
            if evac == "scalar":
                nc.scalar.activation(dst, po[:], AF.Copy)
            else:
                # steady state: all evacuations on VectorE so ScalarE
                # carries only the exps (the exp chain paces the psS
                # rotation; gpsimd can't read PSUM)
                nc.vector.tensor_copy(dst, po[:])

        def oproj_dma(ci):
            qlo, qw = CHUNKS[ci]
            half = ci % 2
            s4g = (qlo % CH) // 128
            nc.sync.dma_start(
                out_e[qlo:qlo + qw, :].rearrange("(s p) c -> p s c", p=128),
                outS[:, half, s4g:s4g + qw // 128, :])

        # flat pipeline over all (chunk, head-pair, sk-group) slots:
        # scores(i), exp/mul(i-1), attn@v(i-3)
        sweeps = [(ci, hp) for ci in range(len(CHUNKS)) for hp in range(2)]
        NS = len(sweeps)
        NSLOT = NS * NG
        sco = {}
        p2s = {}
        accs_by_si = {}
        att_by_ci = {}
        pending_oproj = []
        for slot in range(NSLOT + 3):
            if slot < NSLOT:
                si, g = divmod(slot, NG)
                ci, hp = sweeps[si]
                qlo, qw = CHUNKS[ci]
                if g == 0:
                    if hp == 1:
                        # enqueue the next chunks' q projections one
                        # sweep ahead of first use so they drain well
                        # before the sweep boundary that reads them
                        # (boundary slots are the pipeline's stall point)
                        if qlo == 0:
                            fillers.extend(
                                [lambda m=m: qk_proj_m("q", 2, m)
                                 for m in range(2)])
                        elif qlo == CH:
                            fillers.extend(
                                [lambda m=m: qk_proj_m("q", 3, m)
                                 for m in range(2)])
                    else:
                        att_by_ci[ci] = pA.tile([128, 2, CH], bf16,
                                                name="att", tag="att")
                    accs_by_si[si] = [psA.tile([65, CH], f32, tag="acc",
                                               name=f"acc{j}")
                                      for j in range(2)]
                if slot == 0:
                    drain_filler(2)
                else:
                    drain_filler(3 if slot < 10 else 2)
                sz = GS[g + 1] - GS[g]
                new = [psS.tile([128, T2, CH], f32, tag="sc", name="sc")
                       for j in range(2)]
                # j-major emission: j0's pair finishes two matmuls
                # earlier (j1's completes at the same position), which
                # shortens the scores->exp handoff that paces the
                # ScalarE-saturated sweeps
                for j in range(2):
                    for u in range(sz):
                        t = GS[g] + u
                        nc.tensor.matmul(
                            new[j][:, u, 0:qw],
                            kT[j * 64:(j + 1) * 64, hp,
                               t * 128:(t + 1) * 128],
                            qT[j * 64:(j + 1) * 64, hp, qlo:qlo + qw],
                            start=True, stop=True)
                sco[slot] = new
            if 0 <= slot - 1 < NSLOT:
                sl = slot - 1
                si, g = divmod(sl, NG)
                ci, hp = sweeps[si]
                qlo, qw = CHUNKS[ci]
                nq_c, co = divmod(qlo, CH)
                sz = GS[g + 1] - GS[g]
                cur = []
                for j in range(2):
                    p = pP.tile([128, T2, CH], bf16)
                    nc.scalar.activation(
                        p[:, :sz, 0:qw], sco[sl][j][:, :sz, 0:qw],
                        AF.Exp, scale=SCALE)
                    p2 = pP2.tile([128, T2, CH], bf16)
                    nc.vector.tensor_mul(
                        p2[:, :sz, 0:qw], p[:, :sz, 0:qw],
                        eb[:, nq_c, GS[g]:GS[g] + sz, co:co + qw])
                    cur.append(p2)
                p2s[sl] = cur
                del sco[sl]
            if 0 <= slot - 3 < NSLOT:
                sl = slot - 3
                si, g = divmod(sl, NG)
                ci, hp = sweeps[si]
                qlo, qw = CHUNKS[ci]
                accs = accs_by_si[si]
                sz = GS[g + 1] - GS[g]
                for j in range(2):
                    h = hp * 2 + j
                    for u in range(sz):
                        t = GS[g] + u
                        nc.tensor.matmul(
                            accs[j][:, 0:qw], va[:, t, h, :],
                            p2s[sl][j][:, u, 0:qw],
                            start=(t == 0), stop=(t == ST - 1))
                del p2s[sl]
                if pending_oproj:
                    # o-projection pieces drip two per three slots: PE
                    # filler spread through ~1.5 following sweeps, paced
                    # so the drip sweeps' VectorE load (evacuations +
                    # muls + normalize) stays below the ScalarE exp pace
                    # that sets the slot period. (The att tile's pool
                    # buffer is re-allocated two chunks later, leaving
                    # three slots of margin after the 12-slot drip.)
                    oci, oatt, pidx, tick = pending_oproj[0]
                    if tick % 3 != 2:
                        oproj_piece(oci, oatt, pidx)
                        if pidx == (CHUNKS[oci][1] // 128) * 2 - 1:
                            oproj_dma(oci)
                            pending_oproj.pop(0)
                            del att_by_ci[oci]
                        else:
                            pending_oproj[0] = (oci, oatt, pidx + 1,
                                                tick + 1)
                    else:
                        pending_oproj[0] = (oci, oatt, pidx, tick + 1)
                if g == NG - 1:
                    # sweep complete: normalize this head-pair's rows.
                    # (The rowsum must be staged to SBUF first: the
                    # custom-DVE reciprocal's bitwise seed misreads PSUM.)
                    # In the very last sweep ScalarE is idle (no exps
                    # left), so j1's rowsum copy goes there to shorten
                    # the serial drain chain on VectorE.
                    last_sweep = (si == NS - 1)
                    att = att_by_ci[ci]
                    for j in range(2):
                        rs = pR.tile([1, CH], f32, tag="rs", name="rs")
                        if last_sweep and j == 1:
                            nc.scalar.activation(rs[:, 0:qw],
                                                 accs[j][64:65, 0:qw],
                                                 AF.Copy)
                        else:
                            nc.vector.tensor_copy(rs[:, 0:qw],
                                                  accs[j][64:65, 0:qw])
                        r = pR.tile([1, CH], f32)
                        nc.vector.reciprocal_approx_fast(
                            r[:, 0:qw], rs[:, 0:qw])
                        rb = pRB.tile([64, CH], f32)
                        nc.gpsimd.partition_broadcast(
                            rb[:, 0:qw], r[:, 0:qw])
                        nc.vector.tensor_mul(
                            att[j * 64:(j + 1) * 64, hp, 0:qw],
                            accs[j][0:64, 0:qw], rb[:, 0:qw])
                    del accs_by_si[si]
                    if hp == 1:
                        pending_oproj.append((ci, att_by_ci[ci], 0, 0))
        # drain any remaining o-projection (the last half-chunk): both
        # evac engines are idle here, so evacuations alternate
        # scalar/vector to shorten the psX-rotation chain, and each
        # per-s4 output piece leaves immediately on the sync HWDGE ring
        # (drains instantly, unlike the gpsimd SWDGE queue)
        drain_engines = ["scalar", "vector"]
        for oci, oatt, pidx, _tick in pending_oproj:
            qlo, qw = CHUNKS[oci]
            half = oci % 2
            s4g0 = (qlo % CH) // 128
            for piece in range(pidx, (qw // 128) * 2):
                # odd pieces borrow a bank from the scores pool (idle
                # after the last exp), doubling the po rotation depth so
                # the drain matmuls aren't gated on evacuations two
                # pieces back
                po = None
                if piece % 2 == 1:
                    pos = psS.tile([128, T2, CH], f32, tag="sc",
                                   name="pod")
                    po = pos[:, 0, :]
                oproj_piece(oci, oatt, piece,
                            evac=drain_engines[piece % 2], po=po)
                if piece % 2 == 1:
                    s4l = piece // 2
                    nc.sync.dma_start(
                        out_e[qlo + s4l * 128:qlo + (s4l + 1) * 128, :],
                        outS[:, half, s4g0 + s4l, :])
    nc.compile()
    return nc


def _get_graph(has_qk_bias):
    key = ("nc", has_qk_bias)
    if key not in _graph_cache:
        _graph_cache[key] = _build(has_qk_bias)
    return _graph_cache[key]


def _prepare_in_maps(inputs, has_qk_bias):
    x = np.asarray(inputs["x"], np.float32)
    bm = np.asarray(inputs["binding_matrix"], np.float32)
    Wq = np.asarray(inputs["Wq"], np.float32)
    Wk = np.asarray(inputs["Wk"], np.float32)
    Wv = np.asarray(inputs["Wv"], np.float32)
    Wo = np.asarray(inputs["Wo"], np.float32)
    bq = np.asarray(inputs["bq"], np.float32)
    bk = np.asarray(inputs["bk"], np.float32)

    expbT = np.exp(0.5 * bm.T).astype(BF16)
    xTs = [np.ascontiguousarray(x[b].T).astype(BF16) for b in range(B)]
    in_maps = []
    for c in range(NCORES):
        b, g = divmod(c, 4)
        sl = slice(g * DHC, (g + 1) * DHC)
        m = {
            "xT": xTs[b],
            "wq": np.ascontiguousarray(Wq[:, sl]).astype(BF16),
            "wk": np.ascontiguousarray(Wk[:, sl]).astype(BF16),
            "wv": np.ascontiguousarray(Wv[:, sl]).astype(BF16),
            "wo": np.ascontiguousarray(Wo[sl, :]).astype(BF16),
            "expbT": expbT,
        }
        if has_qk_bias:
            m["bq"] = np.ascontiguousarray(bq[sl]).reshape(DHC, 1)
            m["bk"] = np.ascontiguousarray(bk[sl]).reshape(DHC, 1)
        in_maps.append(m)
    return in_maps


def _install_trace_hooks():
    """The container image's antenv stub lacks axon_hooks; synthesize it so
    run_bass_kernel_spmd(trace=True) can reach the NTFF profiler in
    libaxon_pjrt.so, and neuter the bucket artifact upload."""
    import types

    try:
        from antenv.axon_hooks import get_axon_ntff_profile_hook  # noqa: F401
    except ImportError:
        import antenv

        m = types.ModuleType("antenv.axon_hooks")
        m._hook = None
        m.set_axon_ntff_profile_hook = lambda h: setattr(m, "_hook", h)
        m.get_axon_ntff_profile_hook = lambda: m._hook
        sys.modules["antenv.axon_hooks"] = m
        antenv.axon_hooks = m
        if "/root/.axon_site" not in sys.path:
            sys.path.insert(0, "/root/.axon_site")
        from trn_agent_boot.trn_boot import _ntff_profile_via_ctypes

        m._hook = _ntff_profile_via_ctypes("/opt/axon/libaxon_pjrt.so")
    import concourse.bass_utils as bu

    bu.upload_artifacts = lambda tmpdir: str(tmpdir)


def run(inputs, trace=False, tmpdir=None):
    from concourse.bass_utils import run_bass_kernel_spmd

    if trace:
        _install_trace_hooks()
    bq = np.asarray(inputs["bq"], np.float32)
    bk = np.asarray(inputs["bk"], np.float32)
    has_qk_bias = bool(np.any(bq) or np.any(bk))
    nc = _get_graph(has_qk_bias)
    in_maps = _prepare_in_maps(inputs, has_qk_bias)
    res = run_bass_kernel_spmd(nc, in_maps, list(range(NCORES)), trace=trace,
                               tmpdir=tmpdir)

    bv = np.asarray(inputs["bv"], np.float32)
    bo = np.asarray(inputs["bo"], np.float32)
    Wo = np.asarray(inputs["Wo"], np.float32)
    const_vec = (bv @ Wo + bo).astype(np.float32)

    out = np.empty((B, S, D), np.float32)
    for b in range(B):
        acc = np.zeros((S, D), np.float32)
        for g in range(4):
            acc += np.asarray(res.results[b * 4 + g]["out"], np.float32)
        out[b] = acc + const_vec
    return out, res


def kernel(**inputs):
    out, _ = run(inputs, trace=False)
    return out

